# revision 12
# baseline (speedup 1.0000x reference)
"""Trainium2 Bass kernel for nn_BSN_76218489635087 (segment_reduce).

Computation (reference):
    h = relu-MLP(x[0])            # [2048, 64]
    s = h @ tr_bags               # [2048, 100000]
    col_max = max over rows       # [100000]
    ref_max = segment_max(col_max, tr_mask, 100)
    y_prob = sigmoid(ref_max @ W4 + b4); y_hat = y_prob >= 0.5

Sharding: tr_bags columns (T) split across 8 cores (12544 padded cols each).
Each core computes the full (replicated) MLP producing hT = h.T, duplicated
into both partition halves [128, 2048]. Bags are host-packed [128, 6272] so
consecutive 128-col tiles alternate partition halves (row groups), letting
LDWEIGHTS overlap in-flight MATMULs. Matmuls run in float32r (full fp32
operands, 1 cycle/row for N>=256 vs 4 for plain fp32).

PSUM drain (the bottleneck): per score tile [128, 2048] either
  - DVE reduce_max direct from PSUM, or
  - ACT copies PSUM -> SBUF fp16, then one DVE tensor_tensor_reduce(max,max)
    folds the 2048 fp16 values to the per-column max,
split ~5:3 so ScalarE and VectorE drain concurrently.

Host gathers the 100352 column maxes, does the segment-max + final
100->1 dot + sigmoid.
"""

import sys
import os

for _p in ("/opt/trn_rl_repo", "/root/.axon_site/_ro/pypackages", "/root/.axon_site"):
    if _p not in sys.path and os.path.isdir(_p):
        sys.path.append(_p)

import numpy as np

from concourse import bass, bacc, tile, mybir
from concourse.bass_utils import run_bass_kernel_spmd

# Problem constants (hardcoded per harness contract)
N = 2048          # instances
D = 512           # input features
T = 100000        # reference instance columns
R = 100           # num references (segments)
NCORES = 8
TPC = 12544       # padded columns per core (= 98 * 128); 8*12544 = 100352
NT = TPC // 128   # 98 column-tiles per core

F32 = mybir.dt.float32
F32R = mybir.dt.float32r
F16 = mybir.dt.float16

# Of every 8 score tiles, this many drain via the ACT-copy path (rest DVE).
ACT_TILES = frozenset({0, 1, 2, 4, 6})

USE_ALT = os.environ.get("K_ALT", "0") == "1"      # row-group alternation
USE_SPLIT = os.environ.get("K_SPLIT", "1") == "1"  # ACT/DVE drain split
XSPLIT = int(os.environ.get("K_XSPLIT", "1792"))   # ACT-drained cols per tile


def _mm(nc, out, lhsT, rhs, **kw):
    nc.tensor.matmul(out, lhsT, rhs, **kw)


def _build_program():
    nc = bacc.Bacc("TRN2", target_bir_lowering=False, debug=False, num_devices=NCORES)

    xT_d = nc.dram_tensor("xT", [D, N], F16, kind="ExternalInput")
    w1_d = nc.dram_tensor("w1", [D, 256], F16, kind="ExternalInput")
    w2_d = nc.dram_tensor("w2", [256, 128], F16, kind="ExternalInput")
    w3_d = nc.dram_tensor("w3", [128, 64], F16, kind="ExternalInput")
    b1_d = nc.dram_tensor("b1", [256, 1], F32, kind="ExternalInput")
    b2_d = nc.dram_tensor("b2", [128, 1], F32, kind="ExternalInput")
    b3_d = nc.dram_tensor("b3", [64, 1], F32, kind="ExternalInput")
    bags_shape = [128, TPC // 2] if USE_ALT else [64, TPC]
    bags_d = nc.dram_tensor("bags", bags_shape, F16, kind="ExternalInput")
    out_d = nc.dram_tensor("colmax_out", [128, NT], F32, kind="ExternalOutput")

    relu = mybir.ActivationFunctionType.Relu
    copyf = mybir.ActivationFunctionType.Copy
    amax = mybir.AluOpType.max

    with tile.TileContext(nc) as tc:
        with (
            tc.tile_pool(name="const", bufs=1) as cpool,
            tc.tile_pool(name="scr", bufs=4) as spool,
            tc.tile_pool(name="psum", bufs=2, space="PSUM") as ppool,
        ):
            # ---- load everything ----
            xT_sb = []
            for k in range(4):
                t = cpool.tile([128, N], F16, tag=f"xT{k}", name=f"xT{k}")
                nc.sync.dma_start(t[:], xT_d[128 * k : 128 * (k + 1), :])
                xT_sb.append(t)
            w1_sb = []
            for k in range(4):
                t = cpool.tile([128, 256], F16, tag=f"w1{k}", name=f"w1s{k}")
                nc.sync.dma_start(t[:], w1_d[128 * k : 128 * (k + 1), :])
                w1_sb.append(t)
            w2_sb = []
            for k in range(2):
                t = cpool.tile([128, 128], F16, tag=f"w2{k}", name=f"w2s{k}")
                nc.sync.dma_start(t[:], w2_d[128 * k : 128 * (k + 1), :])
                w2_sb.append(t)
            w3_sb = cpool.tile([128, 64], F16, tag="w3")
            nc.sync.dma_start(w3_sb[:], w3_d[:, :])
            b1_sb = []
            for m in range(2):
                t = cpool.tile([128, 1], F32, tag=f"b1{m}", name=f"b1s{m}")
                nc.sync.dma_start(t[:], b1_d[128 * m : 128 * (m + 1), :])
                b1_sb.append(t)
            b2_sb = cpool.tile([128, 1], F32, tag="b2")
            nc.sync.dma_start(b2_sb[:], b2_d[:, :])
            b3_sb = cpool.tile([64, 1], F32, tag="b3")
            nc.sync.dma_start(b3_sb[:], b3_d[:, :])

            bags_sb = cpool.tile(bags_shape, F16, tag="bags")
            nc.sync.dma_start(bags_sb[:], bags_d[:, :])

            g1_sb = [
                cpool.tile([128, N], F16, tag=f"g1{m}", name=f"g1s{m}")
                for m in range(2)
            ]
            g2_sb = cpool.tile([128, N], F16, tag="g2")
            hT_sb = cpool.tile([128, N], F16, tag="hT")
            colmax_sb = cpool.tile([128, NT], F32, tag="colmax")
            colmax_a = cpool.tile([128, NT], F32, tag="colmaxa")
            colmax_b = cpool.tile([128, NT], F32, tag="colmaxb")

            # ---- layer 1: g1 = relu(W1.T @ xT + b1) -> [256, 2048] as 2 tiles
            for m in range(2):
                ps = ppool.tile([128, N], F32, tag="ps", name=f"psl1{m}")
                for j in range(4):
                    for k in range(4):
                        _mm(
                            nc,
                            ps[:, 512 * j : 512 * (j + 1)],
                            w1_sb[k][:, 128 * m : 128 * (m + 1)],
                            xT_sb[k][:, 512 * j : 512 * (j + 1)],
                            start=(k == 0),
                            stop=(k == 3),
                        )
                nc.scalar.activation(g1_sb[m][:, :], ps[:, :], relu, bias=b1_sb[m][:, :])

            # ---- layer 2: g2 = relu(W2.T @ g1 + b2) -> [128, 2048]
            ps = ppool.tile([128, N], F32, tag="ps", name="psl2")
            for j in range(4):
                for k in range(2):
                    _mm(
                        nc,
                        ps[:, 512 * j : 512 * (j + 1)],
                        w2_sb[k][:, :],
                        g1_sb[k][:, 512 * j : 512 * (j + 1)],
                        start=(k == 0),
                        stop=(k == 1),
                    )
            nc.scalar.activation(g2_sb[:, :], ps[:, :], relu, bias=b2_sb[:, :])

            # ---- layer 3: hT = relu(W3.T @ g2 + b3) -> [64, 2048], then
            #      duplicated into partitions 64:128 for row-group alternation
            ps = ppool.tile([128, N], F32, tag="ps", name="psl3")
            for j in range(4):
                _mm(
                    nc,
                    ps[0:64, 512 * j : 512 * (j + 1)],
                    w3_sb[:, :],
                    g2_sb[:, 512 * j : 512 * (j + 1)],
                    start=True,
                    stop=True,
                )
            nc.scalar.activation(hT_sb[0:64, :], ps[0:64, :], relu, bias=b3_sb[:, :])
            if USE_ALT:
                nc.sync.dma_start(hT_sb[64:128, :], hT_sb[0:64, :])

            # ---- scores: tile i lives in partition half i%2, col block i//2
            for i in range(NT):
                if USE_ALT:
                    half = 64 * (i % 2)
                    lhsT = bags_sb[half : half + 64, 128 * (i // 2) : 128 * (i // 2) + 128]
                else:
                    half = 0
                    lhsT = bags_sb[:, 128 * i : 128 * (i + 1)]
                ps = ppool.tile([128, N], F32, tag="ps", name=f"pss{i}")
                for j in range(4):
                    _mm(
                        nc,
                        ps[:, 512 * j : 512 * (j + 1)],
                        lhsT,
                        hT_sb[half : half + 64, 512 * j : 512 * (j + 1)],
                        start=True,
                        stop=True,
                    )
                if USE_SPLIT:
                    # DVE direct-reduces the last 256 cols straight from PSUM
                    nc.vector.reduce_max(
                        colmax_b[:, i : i + 1], ps[:, XSPLIT:N], axis=mybir.AxisListType.X
                    )
                    # ACT copies the first XSPLIT cols to fp16; DVE max-tree folds them
                    scr = spool.tile([128, XSPLIT], F16, tag="scr", name=f"scr{i}")
                    nc.scalar.activation(scr[:, :], ps[:, 0:XSPLIT], copyf)
                    t1 = spool.tile([128, XSPLIT // 2], F16, tag="t1", name=f"t1_{i}")
                    nc.vector.tensor_max(
                        t1[:, :], scr[:, 0 : XSPLIT // 2], scr[:, XSPLIT // 2 : XSPLIT]
                    )
                    t2 = spool.tile([128, XSPLIT // 4], F16, tag="t2", name=f"t2_{i}")
                    nc.vector.tensor_max(
                        t2[:, :], t1[:, 0 : XSPLIT // 4], t1[:, XSPLIT // 4 : XSPLIT // 2]
                    )
                    nc.vector.reduce_max(
                        colmax_a[:, i : i + 1], t2[:, :], axis=mybir.AxisListType.X
                    )
                else:
                    nc.vector.reduce_max(
                        colmax_sb[:, i : i + 1], ps[:, :], axis=mybir.AxisListType.X
                    )

            if USE_SPLIT:
                nc.vector.tensor_max(colmax_sb[:, :], colmax_a[:, :], colmax_b[:, :])
            nc.sync.dma_start(out_d[:, :], colmax_sb[:])

    nc.compile()
    return nc


_CACHED = {}


def _get_program():
    if "nc" not in _CACHED:
        _CACHED["nc"] = _build_program()
    return _CACHED["nc"]


def _run_device(in_maps, trace=False):
    nc = _get_program()
    try:
        return run_bass_kernel_spmd(nc, in_maps, list(range(NCORES)), trace=trace)
    except ModuleNotFoundError:
        if not trace:
            raise
        return run_bass_kernel_spmd(nc, in_maps, list(range(NCORES)), trace=False)


def _prep_inputs(x, tr_bags, W1, b1, W2, b2, W3, b3):
    xT = np.ascontiguousarray(np.asarray(x, np.float32)[0].T)  # [512, 2048]
    bags = np.asarray(tr_bags, np.float32)
    bags_pad = np.zeros((64, NCORES * TPC), np.float32)
    bags_pad[:, :T] = bags
    base = {
        "xT": xT.astype(np.float16),
        "w1": np.ascontiguousarray(np.asarray(W1, np.float32).astype(np.float16)),
        "w2": np.ascontiguousarray(np.asarray(W2, np.float32).astype(np.float16)),
        "w3": np.ascontiguousarray(np.asarray(W3, np.float32).astype(np.float16)),
        "b1": np.asarray(b1, np.float32).reshape(256, 1).copy(),
        "b2": np.asarray(b2, np.float32).reshape(128, 1).copy(),
        "b3": np.asarray(b3, np.float32).reshape(64, 1).copy(),
    }
    in_maps = []
    for c in range(NCORES):
        shard = bags_pad[:, c * TPC : (c + 1) * TPC]
        if USE_ALT:
            sh = shard.reshape(64, NT, 128)
            packed = np.empty((128, TPC // 2), np.float32)
            # even tiles -> partitions 0:64, odd tiles -> 64:128, col block i//2
            packed[0:64] = sh[:, 0::2, :].reshape(64, -1)
            packed[64:128] = sh[:, 1::2, :].reshape(64, -1)
        else:
            packed = shard
        m = dict(base)
        m["bags"] = np.ascontiguousarray(packed.astype(np.float16))
        in_maps.append(m)
    return in_maps


def _finish_host(colmax, tr_mask, W4, b4):
    tm = np.asarray(tr_mask)
    boundaries = np.searchsorted(tm, np.arange(R + 1))
    ref_max = np.full(R, -np.inf, np.float32)
    nonempty = boundaries[1:] > boundaries[:-1]
    if nonempty.any():
        starts = boundaries[:-1][nonempty]
        ref_max[nonempty] = np.maximum.reduceat(colmax, starts)[: nonempty.sum()]
    z = ref_max.astype(np.float32) @ np.asarray(W4, np.float32) + np.asarray(
        b4, np.float32
    )
    y_prob = (1.0 / (1.0 + np.exp(-z.astype(np.float64)))).astype(np.float32).squeeze()
    y_hat = np.float32(1.0) if y_prob >= 0.5 else np.float32(0.0)
    return np.asarray(y_prob, np.float32), np.asarray(y_hat, np.float32)


def kernel(x, tr_bags, tr_mask, W1, b1, W2, b2, W3, b3, W4, b4, _trace=False):
    in_maps = _prep_inputs(x, tr_bags, W1, b1, W2, b2, W3, b3)
    res = _run_device(in_maps, trace=_trace)
    colmax_parts = []
    for c in range(NCORES):
        cm = res.results[c]["colmax_out"]  # [128, NT]
        colmax_parts.append(np.asarray(cm).T.reshape(-1))  # [TPC], col-major by tile
    colmax = np.concatenate(colmax_parts)[:T]
    out = _finish_host(colmax, tr_mask, W4, b4)
    if _trace:
        return out, res
    return out


# revision 14
# speedup vs baseline: 1.0426x; 1.0426x over previous
"""Trainium2 Bass kernel for nn_BSN_76218489635087 (segment_reduce).

Computation (reference):
    h = relu-MLP(x[0])            # [2048, 64]
    s = h @ tr_bags               # [2048, 100000]
    col_max = max over rows       # [100000]
    ref_max = segment_max(col_max, tr_mask, 100)
    y_prob = sigmoid(ref_max @ W4 + b4); y_hat = y_prob >= 0.5

Sharding: tr_bags columns (T) split across 8 cores (12544 padded cols each).
Each core computes the full (replicated) MLP producing hT = h.T, duplicated
into both partition halves [128, 2048]. Bags are host-packed [128, 6272] so
consecutive 128-col tiles alternate partition halves (row groups), letting
LDWEIGHTS overlap in-flight MATMULs. Matmuls run in float32r (full fp32
operands, 1 cycle/row for N>=256 vs 4 for plain fp32).

PSUM drain (the bottleneck): per score tile [128, 2048] either
  - DVE reduce_max direct from PSUM, or
  - ACT copies PSUM -> SBUF fp16, then one DVE tensor_tensor_reduce(max,max)
    folds the 2048 fp16 values to the per-column max,
split ~5:3 so ScalarE and VectorE drain concurrently.

Host gathers the 100352 column maxes, does the segment-max + final
100->1 dot + sigmoid.
"""

import sys
import os

for _p in ("/opt/trn_rl_repo", "/root/.axon_site/_ro/pypackages", "/root/.axon_site"):
    if _p not in sys.path and os.path.isdir(_p):
        sys.path.append(_p)

import numpy as np

from concourse import bass, bacc, tile, mybir
from concourse.bass_utils import run_bass_kernel_spmd

# Problem constants (hardcoded per harness contract)
N = 2048          # instances
D = 512           # input features
T = 100000        # reference instance columns
R = 100           # num references (segments)
NCORES = 8
TPC = 12544       # padded columns per core (= 98 * 128); 8*12544 = 100352
NT = TPC // 128   # 98 column-tiles per core

F32 = mybir.dt.float32
F32R = mybir.dt.float32r
F16 = mybir.dt.float16

# Of every 8 score tiles, this many drain via the ACT-copy path (rest DVE).
ACT_TILES = frozenset({0, 1, 2, 4, 6})

USE_ALT = os.environ.get("K_ALT", "0") == "1"      # row-group alternation
USE_SPLIT = os.environ.get("K_SPLIT", "1") == "1"  # ACT/DVE drain split
XSPLIT = int(os.environ.get("K_XSPLIT", "1344"))   # ACT-drained cols per tile
TAILENG = os.environ.get("K_TAILENG", "gpsimd")    # gpsimd | vector


def _mm(nc, out, lhsT, rhs, **kw):
    nc.tensor.matmul(out, lhsT, rhs, **kw)


def _build_program():
    nc = bacc.Bacc("TRN2", target_bir_lowering=False, debug=False, num_devices=NCORES)

    xT_d = nc.dram_tensor("xT", [D, N], F16, kind="ExternalInput")
    w1_d = nc.dram_tensor("w1", [D, 256], F16, kind="ExternalInput")
    w2_d = nc.dram_tensor("w2", [256, 128], F16, kind="ExternalInput")
    w3_d = nc.dram_tensor("w3", [128, 64], F16, kind="ExternalInput")
    b1_d = nc.dram_tensor("b1", [256, 1], F32, kind="ExternalInput")
    b2_d = nc.dram_tensor("b2", [128, 1], F32, kind="ExternalInput")
    b3_d = nc.dram_tensor("b3", [64, 1], F32, kind="ExternalInput")
    bags_shape = [128, TPC // 2] if USE_ALT else [64, TPC]
    bags_d = nc.dram_tensor("bags", bags_shape, F16, kind="ExternalInput")
    out_d = nc.dram_tensor("colmax_out", [128, NT], F32, kind="ExternalOutput")

    relu = mybir.ActivationFunctionType.Relu
    copyf = mybir.ActivationFunctionType.Copy
    amax = mybir.AluOpType.max

    with tile.TileContext(nc) as tc:
        with (
            tc.tile_pool(name="const", bufs=1) as cpool,
            tc.tile_pool(name="scr", bufs=4) as spool,
            tc.tile_pool(name="psum", bufs=2, space="PSUM") as ppool,
        ):
            # ---- load everything ----
            xT_sb = []
            for k in range(4):
                t = cpool.tile([128, N], F16, tag=f"xT{k}", name=f"xT{k}")
                nc.sync.dma_start(t[:], xT_d[128 * k : 128 * (k + 1), :])
                xT_sb.append(t)
            w1_sb = []
            for k in range(4):
                t = cpool.tile([128, 256], F16, tag=f"w1{k}", name=f"w1s{k}")
                nc.sync.dma_start(t[:], w1_d[128 * k : 128 * (k + 1), :])
                w1_sb.append(t)
            w2_sb = []
            for k in range(2):
                t = cpool.tile([128, 128], F16, tag=f"w2{k}", name=f"w2s{k}")
                nc.sync.dma_start(t[:], w2_d[128 * k : 128 * (k + 1), :])
                w2_sb.append(t)
            w3_sb = cpool.tile([128, 64], F16, tag="w3")
            nc.sync.dma_start(w3_sb[:], w3_d[:, :])
            b1_sb = []
            for m in range(2):
                t = cpool.tile([128, 1], F32, tag=f"b1{m}", name=f"b1s{m}")
                nc.sync.dma_start(t[:], b1_d[128 * m : 128 * (m + 1), :])
                b1_sb.append(t)
            b2_sb = cpool.tile([128, 1], F32, tag="b2")
            nc.sync.dma_start(b2_sb[:], b2_d[:, :])
            b3_sb = cpool.tile([64, 1], F32, tag="b3")
            nc.sync.dma_start(b3_sb[:], b3_d[:, :])

            bags_sb = cpool.tile(bags_shape, F16, tag="bags")
            nc.sync.dma_start(bags_sb[:], bags_d[:, :])

            g1_sb = [
                cpool.tile([128, N], F16, tag=f"g1{m}", name=f"g1s{m}")
                for m in range(2)
            ]
            g2_sb = cpool.tile([128, N], F16, tag="g2")
            hT_sb = [
                cpool.tile([64, 512], F16, tag=f"hT{j}", name=f"hT{j}")
                for j in range(4)
            ]
            colmax_sb = cpool.tile([128, NT], F32, tag="colmax")

            # ---- layer 1: g1 = relu(W1.T @ xT + b1) -> [256, 2048] as 2 tiles
            for m in range(2):
                ps = ppool.tile([128, N], F32, tag="ps", name=f"psl1{m}")
                for j in range(4):
                    for k in range(4):
                        _mm(
                            nc,
                            ps[:, 512 * j : 512 * (j + 1)],
                            w1_sb[k][:, 128 * m : 128 * (m + 1)],
                            xT_sb[k][:, 512 * j : 512 * (j + 1)],
                            start=(k == 0),
                            stop=(k == 3),
                        )
                nc.scalar.activation(g1_sb[m][:, :], ps[:, :], relu, bias=b1_sb[m][:, :])

            # ---- layer 2: g2 = relu(W2.T @ g1 + b2) -> [128, 2048]
            ps = ppool.tile([128, N], F32, tag="ps", name="psl2")
            for j in range(4):
                for k in range(2):
                    _mm(
                        nc,
                        ps[:, 512 * j : 512 * (j + 1)],
                        w2_sb[k][:, :],
                        g1_sb[k][:, 512 * j : 512 * (j + 1)],
                        start=(k == 0),
                        stop=(k == 1),
                    )
            nc.scalar.activation(g2_sb[:, :], ps[:, :], relu, bias=b2_sb[:, :])

            # ---- layer 3: hT = relu(W3.T @ g2 + b3) -> [64, 2048], then
            #      duplicated into partitions 64:128 for row-group alternation
            ps = ppool.tile([128, N], F32, tag="ps", name="psl3")
            for j in range(4):
                _mm(
                    nc,
                    ps[0:64, 512 * j : 512 * (j + 1)],
                    w3_sb[:, :],
                    g2_sb[:, 512 * j : 512 * (j + 1)],
                    start=True,
                    stop=True,
                )
            for j in range(4):
                nc.scalar.activation(
                    hT_sb[j][:, :], ps[0:64, 512 * j : 512 * (j + 1)], relu,
                    bias=b3_sb[:, :],
                )

            # ---- scores: tile i lives in partition half i%2, col block i//2
            for i in range(NT):
                lhsT = bags_sb[:, 128 * i : 128 * (i + 1)]
                ps = ppool.tile([128, N], F32, tag="ps", name=f"pss{i}")
                for j in range(4):
                    _mm(
                        nc,
                        ps[:, 512 * j : 512 * (j + 1)],
                        lhsT,
                        hT_sb[j][:, :],
                        start=True,
                        stop=True,
                    )
                if USE_SPLIT and (i % 7) < 6:
                    scr = spool.tile([128, N], F16, tag="scr", name=f"scr{i}")
                    nc.scalar.activation(scr[:, :], ps[:, :], copyf)
                    t1 = spool.tile([128, N // 2], F16, tag="t1", name=f"t1_{i}")
                    nc.vector.tensor_max(t1[:, :], scr[:, 0 : N // 2], scr[:, N // 2 : N])
                    t2 = spool.tile([128, N // 4], F16, tag="t2", name=f"t2_{i}")
                    nc.vector.tensor_max(t2[:, :], t1[:, 0 : N // 4], t1[:, N // 4 : N // 2])
                    nc.vector.reduce_max(
                        colmax_sb[:, i : i + 1], t2[:, :], axis=mybir.AxisListType.X
                    )
                else:
                    nc.vector.reduce_max(
                        colmax_sb[:, i : i + 1], ps[:, :], axis=mybir.AxisListType.X
                    )

            nc.sync.dma_start(out_d[:, :], colmax_sb[:])

    nc.compile()
    return nc


_CACHED = {}


def _get_program():
    if "nc" not in _CACHED:
        _CACHED["nc"] = _build_program()
    return _CACHED["nc"]


def _run_device(in_maps, trace=False):
    nc = _get_program()
    try:
        return run_bass_kernel_spmd(nc, in_maps, list(range(NCORES)), trace=trace)
    except ModuleNotFoundError:
        if not trace:
            raise
        return run_bass_kernel_spmd(nc, in_maps, list(range(NCORES)), trace=False)


def _prep_inputs(x, tr_bags, W1, b1, W2, b2, W3, b3):
    xT = np.ascontiguousarray(np.asarray(x, np.float32)[0].T)  # [512, 2048]
    bags = np.asarray(tr_bags, np.float32)
    bags_pad = np.zeros((64, NCORES * TPC), np.float32)
    bags_pad[:, :T] = bags
    base = {
        "xT": xT.astype(np.float16),
        "w1": np.ascontiguousarray(np.asarray(W1, np.float32).astype(np.float16)),
        "w2": np.ascontiguousarray(np.asarray(W2, np.float32).astype(np.float16)),
        "w3": np.ascontiguousarray(np.asarray(W3, np.float32).astype(np.float16)),
        "b1": np.asarray(b1, np.float32).reshape(256, 1).copy(),
        "b2": np.asarray(b2, np.float32).reshape(128, 1).copy(),
        "b3": np.asarray(b3, np.float32).reshape(64, 1).copy(),
    }
    in_maps = []
    for c in range(NCORES):
        shard = bags_pad[:, c * TPC : (c + 1) * TPC]
        if USE_ALT:
            sh = shard.reshape(64, NT, 128)
            packed = np.empty((128, TPC // 2), np.float32)
            # even tiles -> partitions 0:64, odd tiles -> 64:128, col block i//2
            packed[0:64] = sh[:, 0::2, :].reshape(64, -1)
            packed[64:128] = sh[:, 1::2, :].reshape(64, -1)
        else:
            packed = shard
        m = dict(base)
        m["bags"] = np.ascontiguousarray(packed.astype(np.float16))
        in_maps.append(m)
    return in_maps


def _finish_host(colmax, tr_mask, W4, b4):
    tm = np.asarray(tr_mask)
    boundaries = np.searchsorted(tm, np.arange(R + 1))
    ref_max = np.full(R, -np.inf, np.float32)
    nonempty = boundaries[1:] > boundaries[:-1]
    if nonempty.any():
        starts = boundaries[:-1][nonempty]
        ref_max[nonempty] = np.maximum.reduceat(colmax, starts)[: nonempty.sum()]
    z = ref_max.astype(np.float32) @ np.asarray(W4, np.float32) + np.asarray(
        b4, np.float32
    )
    y_prob = (1.0 / (1.0 + np.exp(-z.astype(np.float64)))).astype(np.float32).squeeze()
    y_hat = np.float32(1.0) if y_prob >= 0.5 else np.float32(0.0)
    return np.asarray(y_prob, np.float32), np.asarray(y_hat, np.float32)


def kernel(x, tr_bags, tr_mask, W1, b1, W2, b2, W3, b3, W4, b4, _trace=False):
    in_maps = _prep_inputs(x, tr_bags, W1, b1, W2, b2, W3, b3)
    res = _run_device(in_maps, trace=_trace)
    colmax_parts = []
    for c in range(NCORES):
        cm = res.results[c]["colmax_out"]  # [128, NT]
        colmax_parts.append(np.asarray(cm).T.reshape(-1))  # [TPC], col-major by tile
    colmax = np.concatenate(colmax_parts)[:T]
    out = _finish_host(colmax, tr_mask, W4, b4)
    if _trace:
        return out, res
    return out


# revision 16
# speedup vs baseline: 1.0466x; 1.0039x over previous
"""Trainium2 Bass kernel for nn_BSN_76218489635087 (segment_reduce).

Computation (reference):
    h = relu-MLP(x[0])            # [2048, 64]
    s = h @ tr_bags               # [2048, 100000]
    col_max = max over rows       # [100000]
    ref_max = segment_max(col_max, tr_mask, 100)
    y_prob = sigmoid(ref_max @ W4 + b4); y_hat = y_prob >= 0.5

Sharding: tr_bags columns (T) split across 8 cores (12544 padded cols each).
Each core computes the full (replicated) MLP producing hT = h.T [64, 2048]
(as 4 n-chunk tiles so score matmuls start per chunk). All matmul operands
are fp16 (1 cycle/row on the PE, fp32 PSUM accumulation; plain fp32 lowers
to 2 half-speed passes and float32r measured ~2.5x slower than fp16 on HW).

PSUM drain (the bottleneck - every score element must leave PSUM through
ScalarE or VectorE at 1 elem/lane/cycle): per score tile [128, 2048],
6 of every 7 tiles go ScalarE-copy -> fp16 SBUF -> VectorE max-tree
(tensor_max halvings at the 2x packed fp16 rate, then reduce_max); the
7th is VectorE reduce_max direct from PSUM. This keeps both engines
draining concurrently; PSUM depth (2 tiles of 4 banks) paces the loop.

Host gathers the 100352 column maxes, does the segment-max + final
100->1 dot + sigmoid.
"""

import sys
import os

for _p in ("/opt/trn_rl_repo", "/root/.axon_site/_ro/pypackages", "/root/.axon_site"):
    if _p not in sys.path and os.path.isdir(_p):
        sys.path.append(_p)

import numpy as np

from concourse import bass, bacc, tile, mybir
from concourse.bass_utils import run_bass_kernel_spmd

# Problem constants (hardcoded per harness contract)
N = 2048          # instances
D = 512           # input features
T = 100000        # reference instance columns
R = 100           # num references (segments)
NCORES = 8
TPC = 12544       # padded columns per core (= 98 * 128); 8*12544 = 100352
NT = TPC // 128   # 98 column-tiles per core

F32 = mybir.dt.float32
F32R = mybir.dt.float32r
F16 = mybir.dt.float16

# Of every 8 score tiles, this many drain via the ACT-copy path (rest DVE).
ACT_TILES = frozenset({0, 1, 2, 4, 6})

USE_ALT = os.environ.get("K_ALT", "0") == "1"      # row-group alternation
USE_SPLIT = os.environ.get("K_SPLIT", "1") == "1"  # ACT/DVE drain split
XSPLIT = int(os.environ.get("K_XSPLIT", "1344"))   # ACT-drained cols per tile
TAILENG = os.environ.get("K_TAILENG", "gpsimd")    # gpsimd | vector


def _mm(nc, out, lhsT, rhs, **kw):
    nc.tensor.matmul(out, lhsT, rhs, **kw)


def _build_program():
    nc = bacc.Bacc("TRN2", target_bir_lowering=False, debug=False, num_devices=NCORES)

    xT_d = nc.dram_tensor("xT", [D, N], F16, kind="ExternalInput")
    w1_d = nc.dram_tensor("w1", [D, 256], F16, kind="ExternalInput")
    w2_d = nc.dram_tensor("w2", [256, 128], F16, kind="ExternalInput")
    w3_d = nc.dram_tensor("w3", [128, 64], F16, kind="ExternalInput")
    b1_d = nc.dram_tensor("b1", [256, 1], F32, kind="ExternalInput")
    b2_d = nc.dram_tensor("b2", [128, 1], F32, kind="ExternalInput")
    b3_d = nc.dram_tensor("b3", [64, 1], F32, kind="ExternalInput")
    bags_shape = [128, TPC // 2] if USE_ALT else [64, TPC]
    bags_d = nc.dram_tensor("bags", bags_shape, F16, kind="ExternalInput")
    out_d = nc.dram_tensor("colmax_out", [128, NT], F32, kind="ExternalOutput")

    relu = mybir.ActivationFunctionType.Relu
    copyf = mybir.ActivationFunctionType.Copy
    amax = mybir.AluOpType.max

    with tile.TileContext(nc) as tc:
        with (
            tc.tile_pool(name="const", bufs=1) as cpool,
            tc.tile_pool(name="scr", bufs=4) as spool,
            tc.tile_pool(name="psum", bufs=2, space="PSUM") as ppool,
        ):
            # ---- load everything ----
            xT_sb = []
            for k in range(4):
                t = cpool.tile([128, N], F16, tag=f"xT{k}", name=f"xT{k}")
                nc.sync.dma_start(t[:], xT_d[128 * k : 128 * (k + 1), :])
                xT_sb.append(t)
            w1_sb = []
            for k in range(4):
                t = cpool.tile([128, 256], F16, tag=f"w1{k}", name=f"w1s{k}")
                nc.sync.dma_start(t[:], w1_d[128 * k : 128 * (k + 1), :])
                w1_sb.append(t)
            w2_sb = []
            for k in range(2):
                t = cpool.tile([128, 128], F16, tag=f"w2{k}", name=f"w2s{k}")
                nc.sync.dma_start(t[:], w2_d[128 * k : 128 * (k + 1), :])
                w2_sb.append(t)
            w3_sb = cpool.tile([128, 64], F16, tag="w3")
            nc.sync.dma_start(w3_sb[:], w3_d[:, :])
            b1_sb = []
            for m in range(2):
                t = cpool.tile([128, 1], F32, tag=f"b1{m}", name=f"b1s{m}")
                nc.sync.dma_start(t[:], b1_d[128 * m : 128 * (m + 1), :])
                b1_sb.append(t)
            b2_sb = cpool.tile([128, 1], F32, tag="b2")
            nc.sync.dma_start(b2_sb[:], b2_d[:, :])
            b3_sb = cpool.tile([64, 1], F32, tag="b3")
            nc.sync.dma_start(b3_sb[:], b3_d[:, :])

            bags_sb = cpool.tile(bags_shape, F16, tag="bags")
            nc.sync.dma_start(bags_sb[:], bags_d[:, :])

            g1_sb = [
                cpool.tile([128, N], F16, tag=f"g1{m}", name=f"g1s{m}")
                for m in range(2)
            ]
            g2_sb = cpool.tile([128, N], F16, tag="g2")
            hT_sb = [
                cpool.tile([64, 512], F16, tag=f"hT{j}", name=f"hT{j}")
                for j in range(4)
            ]
            colmax_sb = cpool.tile([128, NT], F32, tag="colmax")

            # ---- layer 1: g1 = relu(W1.T @ xT + b1) -> [256, 2048] as 2 tiles
            for m in range(2):
                ps = ppool.tile([128, N], F32, tag="ps", name=f"psl1{m}")
                for j in range(4):
                    for k in range(4):
                        _mm(
                            nc,
                            ps[:, 512 * j : 512 * (j + 1)],
                            w1_sb[k][:, 128 * m : 128 * (m + 1)],
                            xT_sb[k][:, 512 * j : 512 * (j + 1)],
                            start=(k == 0),
                            stop=(k == 3),
                        )
                nc.scalar.activation(g1_sb[m][:, :], ps[:, :], relu, bias=b1_sb[m][:, :])

            # ---- layer 2: g2 = relu(W2.T @ g1 + b2) -> [128, 2048]
            ps = ppool.tile([128, N], F32, tag="ps", name="psl2")
            for j in range(4):
                for k in range(2):
                    _mm(
                        nc,
                        ps[:, 512 * j : 512 * (j + 1)],
                        w2_sb[k][:, :],
                        g1_sb[k][:, 512 * j : 512 * (j + 1)],
                        start=(k == 0),
                        stop=(k == 1),
                    )
            nc.scalar.activation(g2_sb[:, :], ps[:, :], relu, bias=b2_sb[:, :])

            # ---- layer 3: hT = relu(W3.T @ g2 + b3) -> [64, 2048], then
            #      duplicated into partitions 64:128 for row-group alternation
            ps = ppool.tile([128, N], F32, tag="ps", name="psl3")
            for j in range(4):
                _mm(
                    nc,
                    ps[0:64, 512 * j : 512 * (j + 1)],
                    w3_sb[:, :],
                    g2_sb[:, 512 * j : 512 * (j + 1)],
                    start=True,
                    stop=True,
                )
            for j in range(4):
                nc.scalar.activation(
                    hT_sb[j][:, :], ps[0:64, 512 * j : 512 * (j + 1)], relu,
                    bias=b3_sb[:, :],
                )

            # ---- scores: tile i lives in partition half i%2, col block i//2
            # Tails are emitted one tile late so a direct tile's PSUM reduce
            # sits BEFORE the previous tail in DVE program order - its PSUM
            # slot then frees ~2us earlier and ScalarE never stalls on refill.
            deferred = None

            def emit_tail(scr, i):
                t1 = spool.tile([128, N // 2], F16, tag="t1", name=f"t1_{i}")
                nc.vector.tensor_max(t1[:, :], scr[:, 0 : N // 2], scr[:, N // 2 : N])
                t2 = spool.tile([128, N // 4], F16, tag="t2", name=f"t2_{i}")
                nc.vector.tensor_max(t2[:, :], t1[:, 0 : N // 4], t1[:, N // 4 : N // 2])
                nc.vector.reduce_max(
                    colmax_sb[:, i : i + 1], t2[:, :], axis=mybir.AxisListType.X
                )

            for i in range(NT):
                lhsT = bags_sb[:, 128 * i : 128 * (i + 1)]
                ps = ppool.tile([128, N], F32, tag="ps", name=f"pss{i}")
                for j in range(4):
                    _mm(
                        nc,
                        ps[:, 512 * j : 512 * (j + 1)],
                        lhsT,
                        hT_sb[j][:, :],
                        start=True,
                        stop=True,
                    )
                if USE_SPLIT and (i % 7) < 6:
                    scr = spool.tile([128, N], F16, tag="scr", name=f"scr{i}")
                    nc.scalar.activation(scr[:, :], ps[:, :], copyf)
                    if deferred is not None:
                        emit_tail(*deferred)
                    deferred = (scr, i)
                else:
                    nc.vector.reduce_max(
                        colmax_sb[:, i : i + 1], ps[:, :], axis=mybir.AxisListType.X
                    )
                    if deferred is not None:
                        emit_tail(*deferred)
                        deferred = None
            if deferred is not None:
                emit_tail(*deferred)

            nc.sync.dma_start(out_d[:, :], colmax_sb[:])

    nc.compile()
    return nc


_CACHED = {}


def _get_program():
    if "nc" not in _CACHED:
        _CACHED["nc"] = _build_program()
    return _CACHED["nc"]


def _run_device(in_maps, trace=False):
    nc = _get_program()
    try:
        return run_bass_kernel_spmd(nc, in_maps, list(range(NCORES)), trace=trace)
    except ModuleNotFoundError:
        if not trace:
            raise
        return run_bass_kernel_spmd(nc, in_maps, list(range(NCORES)), trace=False)


def _prep_inputs(x, tr_bags, W1, b1, W2, b2, W3, b3):
    xT = np.ascontiguousarray(np.asarray(x, np.float32)[0].T)  # [512, 2048]
    bags = np.asarray(tr_bags, np.float32)
    bags_pad = np.zeros((64, NCORES * TPC), np.float32)
    bags_pad[:, :T] = bags
    base = {
        "xT": xT.astype(np.float16),
        "w1": np.ascontiguousarray(np.asarray(W1, np.float32).astype(np.float16)),
        "w2": np.ascontiguousarray(np.asarray(W2, np.float32).astype(np.float16)),
        "w3": np.ascontiguousarray(np.asarray(W3, np.float32).astype(np.float16)),
        "b1": np.asarray(b1, np.float32).reshape(256, 1).copy(),
        "b2": np.asarray(b2, np.float32).reshape(128, 1).copy(),
        "b3": np.asarray(b3, np.float32).reshape(64, 1).copy(),
    }
    in_maps = []
    for c in range(NCORES):
        shard = bags_pad[:, c * TPC : (c + 1) * TPC]
        if USE_ALT:
            sh = shard.reshape(64, NT, 128)
            packed = np.empty((128, TPC // 2), np.float32)
            # even tiles -> partitions 0:64, odd tiles -> 64:128, col block i//2
            packed[0:64] = sh[:, 0::2, :].reshape(64, -1)
            packed[64:128] = sh[:, 1::2, :].reshape(64, -1)
        else:
            packed = shard
        m = dict(base)
        m["bags"] = np.ascontiguousarray(packed.astype(np.float16))
        in_maps.append(m)
    return in_maps


def _finish_host(colmax, tr_mask, W4, b4):
    tm = np.asarray(tr_mask)
    boundaries = np.searchsorted(tm, np.arange(R + 1))
    ref_max = np.full(R, -np.inf, np.float32)
    nonempty = boundaries[1:] > boundaries[:-1]
    if nonempty.any():
        starts = boundaries[:-1][nonempty]
        ref_max[nonempty] = np.maximum.reduceat(colmax, starts)[: nonempty.sum()]
    z = ref_max.astype(np.float32) @ np.asarray(W4, np.float32) + np.asarray(
        b4, np.float32
    )
    y_prob = (1.0 / (1.0 + np.exp(-z.astype(np.float64)))).astype(np.float32).squeeze()
    y_hat = np.float32(1.0) if y_prob >= 0.5 else np.float32(0.0)
    return np.asarray(y_prob, np.float32), np.asarray(y_hat, np.float32)


def kernel(x, tr_bags, tr_mask, W1, b1, W2, b2, W3, b3, W4, b4, _trace=False):
    in_maps = _prep_inputs(x, tr_bags, W1, b1, W2, b2, W3, b3)
    res = _run_device(in_maps, trace=_trace)
    colmax_parts = []
    for c in range(NCORES):
        cm = res.results[c]["colmax_out"]  # [128, NT]
        colmax_parts.append(np.asarray(cm).T.reshape(-1))  # [TPC], col-major by tile
    colmax = np.concatenate(colmax_parts)[:T]
    out = _finish_host(colmax, tr_mask, W4, b4)
    if _trace:
        return out, res
    return out


# revision 17
# speedup vs baseline: 1.0480x; 1.0013x over previous
"""Trainium2 Bass kernel for nn_BSN_76218489635087 (segment_reduce).

Computation (reference):
    h = relu-MLP(x[0])            # [2048, 64]
    s = h @ tr_bags               # [2048, 100000]
    col_max = max over rows       # [100000]
    ref_max = segment_max(col_max, tr_mask, 100)
    y_prob = sigmoid(ref_max @ W4 + b4); y_hat = y_prob >= 0.5

Sharding: tr_bags columns (T) split across 8 cores (12544 padded cols each).
Each core computes the full (replicated) MLP producing hT = h.T [64, 2048]
(as 4 n-chunk tiles so score matmuls start per chunk). All matmul operands
are fp16 (1 cycle/row on the PE, fp32 PSUM accumulation; plain fp32 lowers
to 2 half-speed passes and float32r measured ~2.5x slower than fp16 on HW).

PSUM drain (the bottleneck - every score element must leave PSUM through
ScalarE or VectorE at 1 elem/lane/cycle): per score tile [128, 2048],
6 of every 7 tiles go ScalarE-copy -> fp16 SBUF -> VectorE max-tree
(tensor_max halvings at the 2x packed fp16 rate, then reduce_max); the
7th is VectorE reduce_max direct from PSUM. This keeps both engines
draining concurrently; PSUM depth (2 tiles of 4 banks) paces the loop.

Host gathers the 100352 column maxes, does the segment-max + final
100->1 dot + sigmoid.
"""

import sys
import os

for _p in ("/opt/trn_rl_repo", "/root/.axon_site/_ro/pypackages", "/root/.axon_site"):
    if _p not in sys.path and os.path.isdir(_p):
        sys.path.append(_p)

import numpy as np

from concourse import bass, bacc, tile, mybir
from concourse.bass_utils import run_bass_kernel_spmd

# Problem constants (hardcoded per harness contract)
N = 2048          # instances
D = 512           # input features
T = 100000        # reference instance columns
R = 100           # num references (segments)
NCORES = 8
TPC = 12544       # padded columns per core (= 98 * 128); 8*12544 = 100352
NT = TPC // 128   # 98 column-tiles per core

F32 = mybir.dt.float32
F32R = mybir.dt.float32r
F16 = mybir.dt.float16

# Of every 8 score tiles, this many drain via the ACT-copy path (rest DVE).
ACT_TILES = frozenset({0, 1, 2, 4, 6})

USE_ALT = os.environ.get("K_ALT", "0") == "1"      # row-group alternation
USE_SPLIT = os.environ.get("K_SPLIT", "1") == "1"  # ACT/DVE drain split
PDIRECT = int(os.environ.get("K_PDIRECT", "98"))   # ACT-path tiles per 98 (rest DVE-direct)
TAILENG = os.environ.get("K_TAILENG", "gpsimd")    # gpsimd | vector


def _mm(nc, out, lhsT, rhs, **kw):
    nc.tensor.matmul(out, lhsT, rhs, **kw)


def _build_program():
    nc = bacc.Bacc("TRN2", target_bir_lowering=False, debug=False, num_devices=NCORES)

    xT_d = nc.dram_tensor("xT", [D, N], F16, kind="ExternalInput")
    w1_d = nc.dram_tensor("w1", [D, 256], F16, kind="ExternalInput")
    w2_d = nc.dram_tensor("w2", [256, 128], F16, kind="ExternalInput")
    w3_d = nc.dram_tensor("w3", [128, 64], F16, kind="ExternalInput")
    b1_d = nc.dram_tensor("b1", [256, 1], F32, kind="ExternalInput")
    b2_d = nc.dram_tensor("b2", [128, 1], F32, kind="ExternalInput")
    b3_d = nc.dram_tensor("b3", [64, 1], F32, kind="ExternalInput")
    bags_shape = [128, TPC // 2] if USE_ALT else [64, TPC]
    bags_d = nc.dram_tensor("bags", bags_shape, F16, kind="ExternalInput")
    out_d = nc.dram_tensor("colmax_out", [128, NT], F32, kind="ExternalOutput")

    relu = mybir.ActivationFunctionType.Relu
    copyf = mybir.ActivationFunctionType.Copy
    amax = mybir.AluOpType.max

    with tile.TileContext(nc) as tc:
        with (
            tc.tile_pool(name="const", bufs=1) as cpool,
            tc.tile_pool(name="scr", bufs=4) as spool,
            tc.tile_pool(name="psum", bufs=2, space="PSUM") as ppool,
        ):
            # ---- load everything ----
            xT_sb = []
            for k in range(4):
                t = cpool.tile([128, N], F16, tag=f"xT{k}", name=f"xT{k}")
                nc.sync.dma_start(t[:], xT_d[128 * k : 128 * (k + 1), :])
                xT_sb.append(t)
            w1_sb = []
            for k in range(4):
                t = cpool.tile([128, 256], F16, tag=f"w1{k}", name=f"w1s{k}")
                nc.sync.dma_start(t[:], w1_d[128 * k : 128 * (k + 1), :])
                w1_sb.append(t)
            w2_sb = []
            for k in range(2):
                t = cpool.tile([128, 128], F16, tag=f"w2{k}", name=f"w2s{k}")
                nc.sync.dma_start(t[:], w2_d[128 * k : 128 * (k + 1), :])
                w2_sb.append(t)
            w3_sb = cpool.tile([128, 64], F16, tag="w3")
            nc.sync.dma_start(w3_sb[:], w3_d[:, :])
            b1_sb = []
            for m in range(2):
                t = cpool.tile([128, 1], F32, tag=f"b1{m}", name=f"b1s{m}")
                nc.sync.dma_start(t[:], b1_d[128 * m : 128 * (m + 1), :])
                b1_sb.append(t)
            b2_sb = cpool.tile([128, 1], F32, tag="b2")
            nc.sync.dma_start(b2_sb[:], b2_d[:, :])
            b3_sb = cpool.tile([64, 1], F32, tag="b3")
            nc.sync.dma_start(b3_sb[:], b3_d[:, :])

            bags_sb = cpool.tile(bags_shape, F16, tag="bags")
            nc.sync.dma_start(bags_sb[:], bags_d[:, :])

            g1_sb = [
                cpool.tile([128, N], F16, tag=f"g1{m}", name=f"g1s{m}")
                for m in range(2)
            ]
            g2_sb = cpool.tile([128, N], F16, tag="g2")
            hT_sb = [
                cpool.tile([64, 512], F16, tag=f"hT{j}", name=f"hT{j}")
                for j in range(4)
            ]
            colmax_sb = cpool.tile([128, NT], F32, tag="colmax")

            # ---- layer 1: g1 = relu(W1.T @ xT + b1) -> [256, 2048] as 2 tiles
            for m in range(2):
                ps = ppool.tile([128, N], F32, tag="ps", name=f"psl1{m}")
                for j in range(4):
                    for k in range(4):
                        _mm(
                            nc,
                            ps[:, 512 * j : 512 * (j + 1)],
                            w1_sb[k][:, 128 * m : 128 * (m + 1)],
                            xT_sb[k][:, 512 * j : 512 * (j + 1)],
                            start=(k == 0),
                            stop=(k == 3),
                        )
                nc.scalar.activation(g1_sb[m][:, :], ps[:, :], relu, bias=b1_sb[m][:, :])

            # ---- layer 2: g2 = relu(W2.T @ g1 + b2) -> [128, 2048]
            ps = ppool.tile([128, N], F32, tag="ps", name="psl2")
            for j in range(4):
                for k in range(2):
                    _mm(
                        nc,
                        ps[:, 512 * j : 512 * (j + 1)],
                        w2_sb[k][:, :],
                        g1_sb[k][:, 512 * j : 512 * (j + 1)],
                        start=(k == 0),
                        stop=(k == 1),
                    )
            nc.scalar.activation(g2_sb[:, :], ps[:, :], relu, bias=b2_sb[:, :])

            # ---- layer 3: hT = relu(W3.T @ g2 + b3) -> [64, 2048], then
            #      duplicated into partitions 64:128 for row-group alternation
            ps = ppool.tile([128, N], F32, tag="ps", name="psl3")
            for j in range(4):
                _mm(
                    nc,
                    ps[0:64, 512 * j : 512 * (j + 1)],
                    w3_sb[:, :],
                    g2_sb[:, 512 * j : 512 * (j + 1)],
                    start=True,
                    stop=True,
                )
            for j in range(4):
                nc.scalar.activation(
                    hT_sb[j][:, :], ps[0:64, 512 * j : 512 * (j + 1)], relu,
                    bias=b3_sb[:, :],
                )

            # ---- scores: tile i lives in partition half i%2, col block i//2
            # Tails are emitted one tile late so a direct tile's PSUM reduce
            # sits BEFORE the previous tail in DVE program order - its PSUM
            # slot then frees ~2us earlier and ScalarE never stalls on refill.
            deferred = None

            def emit_tail(scr, i):
                t1 = spool.tile([128, N // 2], F16, tag="t1", name=f"t1_{i}")
                nc.vector.tensor_max(t1[:, :], scr[:, 0 : N // 2], scr[:, N // 2 : N])
                t2 = spool.tile([128, N // 4], F16, tag="t2", name=f"t2_{i}")
                nc.vector.tensor_max(t2[:, :], t1[:, 0 : N // 4], t1[:, N // 4 : N // 2])
                nc.vector.reduce_max(
                    colmax_sb[:, i : i + 1], t2[:, :], axis=mybir.AxisListType.X
                )

            for i in range(NT):
                lhsT = bags_sb[:, 128 * i : 128 * (i + 1)]
                ps = ppool.tile([128, N], F32, tag="ps", name=f"pss{i}")
                for j in range(4):
                    _mm(
                        nc,
                        ps[:, 512 * j : 512 * (j + 1)],
                        lhsT,
                        hT_sb[j][:, :],
                        start=True,
                        stop=True,
                    )
                if USE_SPLIT and (i % 98) < PDIRECT:
                    scr = spool.tile([128, N], F16, tag="scr", name=f"scr{i}")
                    nc.scalar.activation(scr[:, :], ps[:, :], copyf)
                    if deferred is not None:
                        emit_tail(*deferred)
                    deferred = (scr, i)
                else:
                    nc.vector.reduce_max(
                        colmax_sb[:, i : i + 1], ps[:, :], axis=mybir.AxisListType.X
                    )
                    if deferred is not None:
                        emit_tail(*deferred)
                        deferred = None
            if deferred is not None:
                emit_tail(*deferred)

            nc.sync.dma_start(out_d[:, :], colmax_sb[:])

    nc.compile()
    return nc


_CACHED = {}


def _get_program():
    if "nc" not in _CACHED:
        _CACHED["nc"] = _build_program()
    return _CACHED["nc"]


def _run_device(in_maps, trace=False):
    nc = _get_program()
    try:
        return run_bass_kernel_spmd(nc, in_maps, list(range(NCORES)), trace=trace)
    except ModuleNotFoundError:
        if not trace:
            raise
        return run_bass_kernel_spmd(nc, in_maps, list(range(NCORES)), trace=False)


def _prep_inputs(x, tr_bags, W1, b1, W2, b2, W3, b3):
    xT = np.ascontiguousarray(np.asarray(x, np.float32)[0].T)  # [512, 2048]
    bags = np.asarray(tr_bags, np.float32)
    bags_pad = np.zeros((64, NCORES * TPC), np.float32)
    bags_pad[:, :T] = bags
    base = {
        "xT": xT.astype(np.float16),
        "w1": np.ascontiguousarray(np.asarray(W1, np.float32).astype(np.float16)),
        "w2": np.ascontiguousarray(np.asarray(W2, np.float32).astype(np.float16)),
        "w3": np.ascontiguousarray(np.asarray(W3, np.float32).astype(np.float16)),
        "b1": np.asarray(b1, np.float32).reshape(256, 1).copy(),
        "b2": np.asarray(b2, np.float32).reshape(128, 1).copy(),
        "b3": np.asarray(b3, np.float32).reshape(64, 1).copy(),
    }
    in_maps = []
    for c in range(NCORES):
        shard = bags_pad[:, c * TPC : (c + 1) * TPC]
        if USE_ALT:
            sh = shard.reshape(64, NT, 128)
            packed = np.empty((128, TPC // 2), np.float32)
            # even tiles -> partitions 0:64, odd tiles -> 64:128, col block i//2
            packed[0:64] = sh[:, 0::2, :].reshape(64, -1)
            packed[64:128] = sh[:, 1::2, :].reshape(64, -1)
        else:
            packed = shard
        m = dict(base)
        m["bags"] = np.ascontiguousarray(packed.astype(np.float16))
        in_maps.append(m)
    return in_maps


def _finish_host(colmax, tr_mask, W4, b4):
    tm = np.asarray(tr_mask)
    boundaries = np.searchsorted(tm, np.arange(R + 1))
    ref_max = np.full(R, -np.inf, np.float32)
    nonempty = boundaries[1:] > boundaries[:-1]
    if nonempty.any():
        starts = boundaries[:-1][nonempty]
        ref_max[nonempty] = np.maximum.reduceat(colmax, starts)[: nonempty.sum()]
    z = ref_max.astype(np.float32) @ np.asarray(W4, np.float32) + np.asarray(
        b4, np.float32
    )
    y_prob = (1.0 / (1.0 + np.exp(-z.astype(np.float64)))).astype(np.float32).squeeze()
    y_hat = np.float32(1.0) if y_prob >= 0.5 else np.float32(0.0)
    return np.asarray(y_prob, np.float32), np.asarray(y_hat, np.float32)


def kernel(x, tr_bags, tr_mask, W1, b1, W2, b2, W3, b3, W4, b4, _trace=False):
    in_maps = _prep_inputs(x, tr_bags, W1, b1, W2, b2, W3, b3)
    res = _run_device(in_maps, trace=_trace)
    colmax_parts = []
    for c in range(NCORES):
        cm = res.results[c]["colmax_out"]  # [128, NT]
        colmax_parts.append(np.asarray(cm).T.reshape(-1))  # [TPC], col-major by tile
    colmax = np.concatenate(colmax_parts)[:T]
    out = _finish_host(colmax, tr_mask, W4, b4)
    if _trace:
        return out, res
    return out


# revision 19
# speedup vs baseline: 1.0550x; 1.0067x over previous
"""Trainium2 Bass kernel for nn_BSN_76218489635087 (segment_reduce).

Computation (reference):
    h = relu-MLP(x[0])            # [2048, 64]
    s = h @ tr_bags               # [2048, 100000]
    col_max = max over rows       # [100000]
    ref_max = segment_max(col_max, tr_mask, 100)
    y_prob = sigmoid(ref_max @ W4 + b4); y_hat = y_prob >= 0.5

Sharding: tr_bags columns (T) split across 8 cores (12544 padded cols each).
Each core computes the full (replicated) MLP producing hT = h.T [64, 2048]
(as 4 n-chunk tiles so score matmuls start per chunk). All matmul operands
are fp16 (1 cycle/row on the PE, fp32 PSUM accumulation; plain fp32 lowers
to 2 half-speed passes and float32r measured ~2.5x slower than fp16 on HW).

PSUM drain (the bottleneck - every score element must leave PSUM through
ScalarE or VectorE at 1 elem/lane/cycle): every score tile [128, 2048] is
ScalarE-copied to fp16 SBUF, then VectorE folds it with a max-tree
(tensor_max halvings at the 2x packed fp16 rate, then reduce_max).
All-ACT measured fastest: mixing in VectorE-direct tiles exposes the
~1.9us PE refill latency as a ScalarE stall of the same size, so the
loop is paced at ~2.0us/tile by the ScalarE copy stream through the
2-deep (4-bank) PSUM pipeline either way, and all-ACT frees VectorE.

Host gathers the 100352 column maxes, does the segment-max + final
100->1 dot + sigmoid.
"""

import sys
import os

for _p in ("/opt/trn_rl_repo", "/root/.axon_site/_ro/pypackages", "/root/.axon_site"):
    if _p not in sys.path and os.path.isdir(_p):
        sys.path.append(_p)

import numpy as np

from concourse import bass, bacc, tile, mybir
from concourse.bass_utils import run_bass_kernel_spmd

# Problem constants (hardcoded per harness contract)
N = 2048          # instances
D = 512           # input features
T = 100000        # reference instance columns
R = 100           # num references (segments)
NCORES = 8
TPC = 12544       # padded columns per core (= 98 * 128); 8*12544 = 100352
NT = TPC // 128   # 98 column-tiles per core

F32 = mybir.dt.float32
F32R = mybir.dt.float32r
F16 = mybir.dt.float16

# Of every 8 score tiles, this many drain via the ACT-copy path (rest DVE).
ACT_TILES = frozenset({0, 1, 2, 4, 6})

USE_ALT = os.environ.get("K_ALT", "0") == "1"      # row-group alternation
USE_SPLIT = os.environ.get("K_SPLIT", "1") == "1"  # ACT/DVE drain split
PDIRECT = int(os.environ.get("K_PDIRECT", "98"))   # ACT-path tiles per 98 (rest DVE-direct)
TAILENG = os.environ.get("K_TAILENG", "gpsimd")    # gpsimd | vector


def _mm(nc, out, lhsT, rhs, **kw):
    nc.tensor.matmul(out, lhsT, rhs, **kw)


def _build_program():
    nc = bacc.Bacc("TRN2", target_bir_lowering=False, debug=False, num_devices=NCORES)

    xT_d = nc.dram_tensor("xT", [D, N], F16, kind="ExternalInput")
    w1_d = nc.dram_tensor("w1", [D, 256], F16, kind="ExternalInput")
    w2_d = nc.dram_tensor("w2", [256, 128], F16, kind="ExternalInput")
    w3_d = nc.dram_tensor("w3", [128, 64], F16, kind="ExternalInput")
    b1_d = nc.dram_tensor("b1", [256, 1], F32, kind="ExternalInput")
    b2_d = nc.dram_tensor("b2", [128, 1], F32, kind="ExternalInput")
    b3_d = nc.dram_tensor("b3", [64, 1], F32, kind="ExternalInput")
    bags_shape = [128, TPC // 2] if USE_ALT else [64, TPC]
    bags_d = nc.dram_tensor("bags", bags_shape, F16, kind="ExternalInput")
    out_d = nc.dram_tensor("colmax_out", [128, NT], F32, kind="ExternalOutput")

    relu = mybir.ActivationFunctionType.Relu
    copyf = mybir.ActivationFunctionType.Copy
    amax = mybir.AluOpType.max

    with tile.TileContext(nc) as tc:
        with (
            tc.tile_pool(name="const", bufs=1) as cpool,
            tc.tile_pool(name="scr", bufs=4) as spool,
            tc.tile_pool(name="psum", bufs=2, space="PSUM") as ppool,
        ):
            # ---- load everything ----
            xT_sb = []
            for k in range(4):
                t = cpool.tile([128, N], F16, tag=f"xT{k}", name=f"xT{k}")
                nc.sync.dma_start(t[:], xT_d[128 * k : 128 * (k + 1), :])
                xT_sb.append(t)
            w1_sb = []
            for k in range(4):
                t = cpool.tile([128, 256], F16, tag=f"w1{k}", name=f"w1s{k}")
                nc.sync.dma_start(t[:], w1_d[128 * k : 128 * (k + 1), :])
                w1_sb.append(t)
            w2_sb = []
            for k in range(2):
                t = cpool.tile([128, 128], F16, tag=f"w2{k}", name=f"w2s{k}")
                nc.sync.dma_start(t[:], w2_d[128 * k : 128 * (k + 1), :])
                w2_sb.append(t)
            w3_sb = cpool.tile([128, 64], F16, tag="w3")
            nc.sync.dma_start(w3_sb[:], w3_d[:, :])
            b1_sb = []
            for m in range(2):
                t = cpool.tile([128, 1], F32, tag=f"b1{m}", name=f"b1s{m}")
                nc.sync.dma_start(t[:], b1_d[128 * m : 128 * (m + 1), :])
                b1_sb.append(t)
            b2_sb = cpool.tile([128, 1], F32, tag="b2")
            nc.sync.dma_start(b2_sb[:], b2_d[:, :])
            b3_sb = cpool.tile([64, 1], F32, tag="b3")
            nc.sync.dma_start(b3_sb[:], b3_d[:, :])

            bags_sb = cpool.tile(bags_shape, F16, tag="bags")
            nc.sync.dma_start(bags_sb[:], bags_d[:, :])

            g1_sb = [
                cpool.tile([128, N], F16, tag=f"g1{m}", name=f"g1s{m}")
                for m in range(2)
            ]
            g2_sb = cpool.tile([128, N], F16, tag="g2")
            hT_sb = [
                cpool.tile([64, 512], F16, tag=f"hT{j}", name=f"hT{j}")
                for j in range(4)
            ]
            colmax_sb = cpool.tile([128, NT], F32, tag="colmax")

            # ---- layer 1: g1 = relu(W1.T @ xT + b1) -> [256, 2048] as 2 tiles
            for m in range(2):
                ps = ppool.tile([128, N], F32, tag="ps", name=f"psl1{m}")
                for j in range(4):
                    for k in range(4):
                        _mm(
                            nc,
                            ps[:, 512 * j : 512 * (j + 1)],
                            w1_sb[k][:, 128 * m : 128 * (m + 1)],
                            xT_sb[k][:, 512 * j : 512 * (j + 1)],
                            start=(k == 0),
                            stop=(k == 3),
                        )
                if m == 0:
                    nc.scalar.activation(
                        g1_sb[m][:, :], ps[:, :], relu, bias=b1_sb[m][:, :]
                    )
                else:
                    # relu(x + b) on VectorE so both L1 relus run concurrently
                    nc.vector.tensor_scalar(
                        out=g1_sb[m][:, :], in0=ps[:, :],
                        scalar1=b1_sb[m][:, :], scalar2=0.0,
                        op0=amax if False else mybir.AluOpType.add, op1=amax,
                    )

            # ---- layer 2: g2 = relu(W2.T @ g1 + b2) -> [128, 2048]
            ps = ppool.tile([128, N], F32, tag="ps", name="psl2")
            for j in range(4):
                for k in range(2):
                    _mm(
                        nc,
                        ps[:, 512 * j : 512 * (j + 1)],
                        w2_sb[k][:, :],
                        g1_sb[k][:, 512 * j : 512 * (j + 1)],
                        start=(k == 0),
                        stop=(k == 1),
                    )
            nc.scalar.activation(
                g2_sb[:, 0:1024], ps[:, 0:1024], relu, bias=b2_sb[:, :]
            )
            nc.vector.tensor_scalar(
                out=g2_sb[:, 1024:2048], in0=ps[:, 1024:2048],
                scalar1=b2_sb[:, :], scalar2=0.0,
                op0=mybir.AluOpType.add, op1=amax,
            )

            # ---- layer 3: hT = relu(W3.T @ g2 + b3) -> [64, 2048], then
            #      duplicated into partitions 64:128 for row-group alternation
            ps = ppool.tile([128, N], F32, tag="ps", name="psl3")
            for j in range(4):
                _mm(
                    nc,
                    ps[0:64, 512 * j : 512 * (j + 1)],
                    w3_sb[:, :],
                    g2_sb[:, 512 * j : 512 * (j + 1)],
                    start=True,
                    stop=True,
                )
            for j in range(4):
                nc.scalar.activation(
                    hT_sb[j][:, :], ps[0:64, 512 * j : 512 * (j + 1)], relu,
                    bias=b3_sb[:, :],
                )

            # ---- scores: tile i lives in partition half i%2, col block i//2
            # Tails are emitted one tile late so a direct tile's PSUM reduce
            # sits BEFORE the previous tail in DVE program order - its PSUM
            # slot then frees ~2us earlier and ScalarE never stalls on refill.
            deferred = None

            def emit_tail(scr, i):
                t1 = spool.tile([128, N // 2], F16, tag="t1", name=f"t1_{i}")
                nc.vector.tensor_max(t1[:, :], scr[:, 0 : N // 2], scr[:, N // 2 : N])
                t2 = spool.tile([128, N // 4], F16, tag="t2", name=f"t2_{i}")
                nc.vector.tensor_max(t2[:, :], t1[:, 0 : N // 4], t1[:, N // 4 : N // 2])
                nc.vector.reduce_max(
                    colmax_sb[:, i : i + 1], t2[:, :], axis=mybir.AxisListType.X
                )

            for i in range(NT):
                lhsT = bags_sb[:, 128 * i : 128 * (i + 1)]
                ps = ppool.tile([128, N], F32, tag="ps", name=f"pss{i}")
                for j in range(4):
                    _mm(
                        nc,
                        ps[:, 512 * j : 512 * (j + 1)],
                        lhsT,
                        hT_sb[j][:, :],
                        start=True,
                        stop=True,
                    )
                if USE_SPLIT and (i % 98) < PDIRECT:
                    scr = spool.tile([128, N], F16, tag="scr", name=f"scr{i}")
                    nc.scalar.activation(scr[:, :], ps[:, :], copyf)
                    if deferred is not None:
                        emit_tail(*deferred)
                    deferred = (scr, i)
                else:
                    nc.vector.reduce_max(
                        colmax_sb[:, i : i + 1], ps[:, :], axis=mybir.AxisListType.X
                    )
                    if deferred is not None:
                        emit_tail(*deferred)
                        deferred = None
            if deferred is not None:
                emit_tail(*deferred)

            nc.sync.dma_start(out_d[:, :], colmax_sb[:])

    nc.compile()
    return nc


_CACHED = {}


def _get_program():
    if "nc" not in _CACHED:
        _CACHED["nc"] = _build_program()
    return _CACHED["nc"]


def _run_device(in_maps, trace=False):
    nc = _get_program()
    try:
        return run_bass_kernel_spmd(nc, in_maps, list(range(NCORES)), trace=trace)
    except ModuleNotFoundError:
        if not trace:
            raise
        return run_bass_kernel_spmd(nc, in_maps, list(range(NCORES)), trace=False)


def _prep_inputs(x, tr_bags, W1, b1, W2, b2, W3, b3):
    xT = np.ascontiguousarray(np.asarray(x, np.float32)[0].T)  # [512, 2048]
    bags = np.asarray(tr_bags, np.float32)
    bags_pad = np.zeros((64, NCORES * TPC), np.float32)
    bags_pad[:, :T] = bags
    base = {
        "xT": xT.astype(np.float16),
        "w1": np.ascontiguousarray(np.asarray(W1, np.float32).astype(np.float16)),
        "w2": np.ascontiguousarray(np.asarray(W2, np.float32).astype(np.float16)),
        "w3": np.ascontiguousarray(np.asarray(W3, np.float32).astype(np.float16)),
        "b1": np.asarray(b1, np.float32).reshape(256, 1).copy(),
        "b2": np.asarray(b2, np.float32).reshape(128, 1).copy(),
        "b3": np.asarray(b3, np.float32).reshape(64, 1).copy(),
    }
    in_maps = []
    for c in range(NCORES):
        shard = bags_pad[:, c * TPC : (c + 1) * TPC]
        if USE_ALT:
            sh = shard.reshape(64, NT, 128)
            packed = np.empty((128, TPC // 2), np.float32)
            # even tiles -> partitions 0:64, odd tiles -> 64:128, col block i//2
            packed[0:64] = sh[:, 0::2, :].reshape(64, -1)
            packed[64:128] = sh[:, 1::2, :].reshape(64, -1)
        else:
            packed = shard
        m = dict(base)
        m["bags"] = np.ascontiguousarray(packed.astype(np.float16))
        in_maps.append(m)
    return in_maps


def _finish_host(colmax, tr_mask, W4, b4):
    tm = np.asarray(tr_mask)
    boundaries = np.searchsorted(tm, np.arange(R + 1))
    ref_max = np.full(R, -np.inf, np.float32)
    nonempty = boundaries[1:] > boundaries[:-1]
    if nonempty.any():
        starts = boundaries[:-1][nonempty]
        ref_max[nonempty] = np.maximum.reduceat(colmax, starts)[: nonempty.sum()]
    z = ref_max.astype(np.float32) @ np.asarray(W4, np.float32) + np.asarray(
        b4, np.float32
    )
    y_prob = (1.0 / (1.0 + np.exp(-z.astype(np.float64)))).astype(np.float32).squeeze()
    y_hat = np.float32(1.0) if y_prob >= 0.5 else np.float32(0.0)
    return np.asarray(y_prob, np.float32), np.asarray(y_hat, np.float32)


def kernel(x, tr_bags, tr_mask, W1, b1, W2, b2, W3, b3, W4, b4, _trace=False):
    in_maps = _prep_inputs(x, tr_bags, W1, b1, W2, b2, W3, b3)
    res = _run_device(in_maps, trace=_trace)
    colmax_parts = []
    for c in range(NCORES):
        cm = res.results[c]["colmax_out"]  # [128, NT]
        colmax_parts.append(np.asarray(cm).T.reshape(-1))  # [TPC], col-major by tile
    colmax = np.concatenate(colmax_parts)[:T]
    out = _finish_host(colmax, tr_mask, W4, b4)
    if _trace:
        return out, res
    return out
